# revision 18
# baseline (speedup 1.0000x reference)
"""Trainium2 Bass kernel v6 for nn_AttentionLayer.

Math (vocab-sharded across 8 cores, VS=6400 columns each):
    out[b, v] = occ[b, v] * leaky_relu(t[v] + s[b]),
    t = table_shard @ a_w   (PE, fp16 in / f32 PSUM, t replicated over partitions)
    s = attr_emb @ a_a      (host side: 65K MACs, loaded as a [128,2] bias)

Engine plan:
  PE   : t via matmul with column-replicated a_w (fp16, 1 cyc/row), PSUM 3-deep
  ACT  : u = prelu(t + s[b], alpha=0.2) fp16 out — one pass, per-partition bias
  DVE  : mask mult h=0 tiles: o = u * occ (fp16 x int8 -> fp16)
  POOL : mask mult h=1 tiles
  DMA  : inputs on the Scalar HWDGE queue, interleaved between ACT ops in
         program order so neither starves; outputs (fp16, host upcasts to
         f32) exclusively on the Sync queue.
"""

import numpy as np

import concourse.bass as bass
import concourse.tile as tile
from concourse import bacc, mybir
from concourse.bass_utils import run_bass_kernel_spmd

B = 256
L = 512
V = 50257
DW = 256
DA = 256
ALPHA = 0.2

NCORES = 8
VS = 6400          # vocab span per core
SW = 1280          # strip width
NS = VS // SW      # 5 strips

_CACHE = {}


def _build():
    if "nc" in _CACHE:
        return _CACHE["nc"]
    f32 = mybir.dt.float32
    f16 = mybir.dt.float16
    i8 = mybir.dt.int8

    nc = bacc.Bacc("TRN2", target_bir_lowering=False, debug=False)
    tblT = nc.declare_dram_parameter("tblT", [DW, VS], f16, isOutput=False)
    occ = nc.declare_dram_parameter("occ", [B, VS], i8, isOutput=False)
    awbT = nc.declare_dram_parameter("awbT", [128, 2 * 128], f16, isOutput=False)
    sbias = nc.declare_dram_parameter("sbias", [128, 2], f32, isOutput=False)
    out = nc.declare_dram_parameter("out", [B, VS], f16, isOutput=True)

    with tile.TileContext(nc) as tc:
        with (
            tc.tile_pool(name="sb", bufs=1) as sb,
            tc.tile_pool(name="tp", bufs=NS) as tp,
            tc.tile_pool(name="bk", bufs=8) as bk,
            tc.tile_pool(name="pst", bufs=2, space="PSUM") as pst,
        ):
            # all inputs on the Scalar HWDGE queue, in need-order, so the
            # first table strip completes without sharing DMA bandwidth;
            # outputs get the Sync queue to themselves
            awb_t = sb.tile([128, 2 * 128], f16, tag="awb")
            s_sb = sb.tile([128, 2], f32, tag="s")

            # occupancy mask: one persistent tile, two loads (first covers
            # strips 0-1 so early masks don't wait on the full transfer)
            m8 = sb.tile([128, 2 * VS], i8, tag="m8")
            m8v = m8[:].rearrange("p (h v) -> p h v", v=VS)
            occv = occ.ap().rearrange("(h p) v -> p h v", p=128)

            tblv = tblT.ap().rearrange("(dh p) v -> p dh v", p=128)

            def load_tbl(si, eng):
                cs = slice(si * SW, (si + 1) * SW)
                tTt = tp.tile([128, 2 * SW], f16, tag="tblT")
                eng.dma_start(
                    tTt[:].rearrange("p (dh v) -> p dh v", v=SW),
                    tblv[:, :, cs],
                )
                return tTt

            # single Sync DMA queue, strict need-order; Scalar engine is kept
            # free so ACT ops are never queued behind dispatches
            tTts = {}
            tTts[0] = load_tbl(0, nc.sync)
            nc.sync.dma_start(awb_t[:], awbT.ap())
            nc.sync.dma_start(s_sb[:], sbias.ap())
            nc.sync.dma_start(m8v[:, :, 0 : 2 * SW], occv[:, :, 0 : 2 * SW])
            tTts[1] = load_tbl(1, nc.sync)
            tTts[2] = load_tbl(2, nc.sync)
            nc.sync.dma_start(m8v[:, :, 2 * SW : VS], occv[:, :, 2 * SW : VS])
            tTts[3] = load_tbl(3, nc.sync)
            tTts[4] = load_tbl(4, nc.sync)
            pending = []

            # ---- per strip ----
            for si in range(NS):
                tTt = tTts[si]
                pt = pst.tile([128, SW], f32, tag="pt")
                for dh in range(2):
                    for n0, n1 in ((0, 512), (512, 1024), (1024, SW)):
                        nc.tensor.matmul(
                            pt[:, n0:n1],
                            lhsT=awb_t[:, dh * 128 : (dh + 1) * 128],
                            rhs=tTt[:, dh * SW + n0 : dh * SW + n1],
                            start=(dh == 0),
                            stop=(dh == 1),
                        )
                for h in range(2):
                    rows = slice(h * 128, (h + 1) * 128)
                    cs = slice(si * SW, (si + 1) * SW)
                    # u = leaky_relu(t + s[b]) in one ACT pass (Prelu honors
                    # alpha; Lrelu's slope is hard-baked to 0.01)
                    u = bk.tile([128, SW], f16, tag="u")
                    nc.scalar.activation(
                        u[:],
                        pt[:],
                        mybir.ActivationFunctionType.Prelu,
                        bias=s_sb[:, h : h + 1],
                        scale=1.0,
                        alpha=ALPHA,
                    )
                    if pending:
                        pending.pop(0)()
                    o = bk.tile([128, SW], f16, tag="o")
                    meng = nc.vector if h == 1 else nc.gpsimd
                    meng.tensor_tensor(
                        out=o[:],
                        in0=u[:],
                        in1=m8v[:, h, cs],
                        op=mybir.AluOpType.mult,
                    )
                    nc.sync.dma_start(out.ap()[rows, cs], o[:])

    nc.compile()
    _CACHE["nc"] = nc
    return nc


def _prep_inputs(words, word_emb_table, attr_emb, a):
    words = np.ascontiguousarray(words).astype(np.int64)
    wet = np.ascontiguousarray(word_emb_table, dtype=np.float32)
    attr = np.ascontiguousarray(attr_emb, dtype=np.float32)
    a = np.ascontiguousarray(a, dtype=np.float32).reshape(-1)

    # awbT[p, dh*128+m] = a_w[dh*128+p]
    awbT = np.empty((128, 2 * 128), dtype=np.float16)
    for dh in range(2):
        awbT[:, dh * 128 : (dh + 1) * 128] = np.repeat(
            a[dh * 128 : (dh + 1) * 128, None].astype(np.float16), 128, axis=1
        )
    awbT = np.ascontiguousarray(awbT)

    # s[b] = attr_emb[b] @ a_a; sbias[p, h] = s[h*128+p]
    s = attr @ a[DW:]
    sbias = np.ascontiguousarray(s.reshape(2, 128).T.astype(np.float32))

    tblpad = np.zeros((NCORES * VS, DW), dtype=np.float32)
    tblpad[:V] = wet
    tblT_full = np.ascontiguousarray(tblpad.T.astype(np.float16))

    occ_full = np.zeros((B, NCORES * VS), dtype=np.int8)
    rows = np.repeat(np.arange(B), L)
    occ_full[rows, words.reshape(-1)] = 1

    in_maps = []
    for i in range(NCORES):
        in_maps.append(
            {
                "tblT": np.ascontiguousarray(tblT_full[:, i * VS : (i + 1) * VS]),
                "occ": np.ascontiguousarray(occ_full[:, i * VS : (i + 1) * VS]),
                "awbT": awbT,
                "sbias": sbias,
            }
        )
    return in_maps


def kernel(words, word_emb_table, attr_emb, a, _trace=False, **_kw):
    nc = _build()
    in_maps = _prep_inputs(words, word_emb_table, attr_emb, a)
    res = run_bass_kernel_spmd(nc, in_maps, list(range(NCORES)), trace=_trace)
    full = np.concatenate(
        [res.results[i]["out"] for i in range(NCORES)], axis=1
    )
    out = np.ascontiguousarray(full[:, :V].astype(np.float32))
    if _trace:
        return out, res
    return out
